# revision 41
# baseline (speedup 1.0000x reference)
"""Trainium2 Bass kernel for nn_ChannelAttGatedGRUCell.

Reference computation (per batch element b):
    xh = concat([x, h], -1)                                  # (C, 2048)
    r = attn(xh; Wq_r, Wk_r, Wv_r); z = attn(xh; ...z)       # (C, 1024)
    reset = sigmoid(r); update = sigmoid(z)
    xhr = concat([x, h*reset], -1)
    n = attn(xhr; ...n)
    new = (1-update)*h + update*tanh(n)
    out = LayerNorm(new) * gamma + beta

Sharding: data-parallel over batch B=64 across 8 cores (8 per core);
weights replicated.  The host pre-transposes x/h to feature-major and
pre-casts each tensor to the precision its consumers need.

Mixed precision (absmax-rel budget 2e-2; measured 1.62e-2 on HW, CPU
simulation of the exact quantization dataflow predicted 1.78e-2):
  - fp8 e4m3 + DoubleRow perf mode (2 contraction chunks per
    instruction via [128, 2, N] operands, 2x ALU rate) for: the V
    projections, the attn@V contractions, and the x-half of the Q/K
    projections (x is quantized once on the host, shared by all).
  - bf16 (1 cycle/row) for the h-half of the Q/K projections and the
    score matmuls: CPU simulation shows the score path dominates the
    error budget, so it keeps ~8 mantissa bits.  The h-half fp8+bf16
    accumulate into one PSUM group (measured: no mode-switch penalty).
  - exp() is emitted straight from PSUM with a -2.0 bias shift so the
    fp8 attention weights stay inside TRN e4m3's +-240 normal range;
    the softmax denominator sums the *quantized* weights, so
    normalization is exact w.r.t. the fp8 rounding.
  - Final gating + LayerNorm in fp32 (update gate held in SBUF bf16).

On-device dataflow per batch element (layouts avoid all transposes):
    Q_T[d,c]  = Wq8[k,d].T @ x8[k,c] + Wq[k,d].T @ hT[k,c]
    K_T[d,c]  = (same split as Q_T)
    V[e,d]    = x8/h8[k,e].T @ Wv8[k,d]    (fp8 DR; Wv8 resident, 6MB)
    S_T[e,c]  = K_T[d,e].T @ Q_T[d,c]      (bf16)
    E_T[e,c]  = exp(S_T/sqrt(dqk) - 2)     (ScalarE, PSUM -> fp8 SBUF)
    s[c]      = ones.T @ E_T               (softmax denom via matmul)
    U[c,d]    = E_T[e,c].T @ V[e,d]        (fp8 DR; z/n gates)
    U_rT[d,c] = V[e,d].T @ E_T[e,c]        (fp8 DR; r gate)
    1/s folds into the ScalarE sigmoid/tanh `scale` operand (z/n) or
    into one DVE rescale of U_rT (r gate).

Gate order is (z, r, n): after z no longer needs h, the r gate's reset
multiply overwrites the bf16 h tile in place with (h*reset) and then
re-quantizes it over the fp8 h tile, which the n gate reads as the
second half of xhr.  The z update gate stays in SBUF as bf16.  PSUM
banks alternate per batch element / e-tile inside every accumulation
phase (consecutive same-bank matmuls cannot pipeline), and the V
phase runs in 4-bank groups so its drains overlap the next group.
"""

import numpy as np

B, C, D_IN, D_H, D_QK = 64, 512, 1024, 1024, 512
DIN2 = D_IN + D_H
N_CORES = 8
NB = B // N_CORES          # batch elements per core
EPS = 1e-5
P = 128
KC = DIN2 // P             # 16 contraction chunks over concat features
KX = D_IN // P             # 8  (x-half chunks; h-half is chunks KX..KC-1)
KP = KC // 2               # 8  DoubleRow chunk-pairs over concat features
KXP = KX // 2              # 4  DoubleRow pairs in the x half
CT = C // P                # 4  c-tiles (tokens)
ET = CT                    # 4  e-tiles (tokens, attended-over axis)
EP = ET // 2               # 2  DoubleRow e-tile pairs
DTQ = D_QK // P            # 4  d-tiles of q/k head dim
DTH = D_H // P             # 8  d-tiles of value dim
FD = 512                   # matmul moving free dim (fp32 PSUM bank)
NDH = D_H // FD            # 2  d-halves of value dim
SM_SCALE = float(1.0 / np.sqrt(D_QK))
SM_SHIFT = -2.0            # exp bias shift keeping fp8 E in normal range

_CACHE = {}


def _build(nb, use_bias, use_gb, repeat=1):
    """Build + compile the per-core Bass program for nb batch elements.

    repeat>1 wraps the body in a hardware For_i loop that recomputes the
    same outputs; used only to measure per-iteration HW time."""
    import contextlib

    import concourse.bacc as bacc
    import concourse.bass as bass
    import concourse.mybir as mybir
    import concourse.tile as tile

    f32 = mybir.dt.float32
    bf16 = mybir.dt.bfloat16
    f8 = mybir.dt.float8e4
    Alu = mybir.AluOpType
    Act = mybir.ActivationFunctionType
    DR = mybir.MatmulPerfMode.DoubleRow

    nc = bacc.Bacc("TRN2", target_bir_lowering=False, debug=False)

    # ---- DRAM I/O ----
    hTb_t = nc.dram_tensor("hTb", [nb, D_H, C], bf16, kind="ExternalInput")
    x8_t = nc.dram_tensor("x8", [nb, D_IN, C], f8, kind="ExternalInput")
    h8_t = nc.dram_tensor("h8", [nb, D_H, C], f8, kind="ExternalInput")
    hN_t = nc.dram_tensor("hN", [nb, C, D_H], f32, kind="ExternalInput")
    w_t = {}
    w8_t = {}
    wv_t = {}
    b_t = {}
    for g in ("r", "z", "n"):
        # h-half of the q/k weights stays bf16; the x-half is fp8 and runs
        # in DoubleRow mode (the x input is quantized once, shared with the
        # V projections)
        w_t[g, "q"] = nc.dram_tensor(f"Wq_{g}", [D_H, D_QK], bf16, kind="ExternalInput")
        w_t[g, "k"] = nc.dram_tensor(f"Wk_{g}", [D_H, D_QK], bf16, kind="ExternalInput")
        w8_t[g, "q"] = nc.dram_tensor(f"Wq8_{g}", [D_IN, D_QK], f8, kind="ExternalInput")
        w8_t[g, "k"] = nc.dram_tensor(f"Wk8_{g}", [D_IN, D_QK], f8, kind="ExternalInput")
        wv_t[g] = nc.dram_tensor(f"Wv8_{g}", [DIN2, D_H], f8, kind="ExternalInput")
        b_t[g, "q"] = nc.dram_tensor(f"bq_{g}", [D_QK], f32, kind="ExternalInput")
        b_t[g, "k"] = nc.dram_tensor(f"bk_{g}", [D_QK], f32, kind="ExternalInput")
        b_t[g, "v"] = nc.dram_tensor(f"bv_{g}", [D_H], f32, kind="ExternalInput")
    gamma_t = nc.dram_tensor("gamma", [D_H], f32, kind="ExternalInput")
    beta_t = nc.dram_tensor("beta", [D_H], f32, kind="ExternalInput")
    out_t = nc.dram_tensor("out", [nb, C, D_H], f32, kind="ExternalOutput")

    hTb_ap = hTb_t.ap()
    x8_ap = x8_t.ap()
    h8_ap = h8_t.ap()
    hN_ap = hN_t.ap()
    out_ap = out_t.ap()
    # feature-major [128, ko, free] views of weights
    wqk_view = {k: v.ap().rearrange("(ko p) d -> p ko d", p=P) for k, v in w_t.items()}
    w8qk_view = {k: v.ap().rearrange("(ko p) d -> p ko d", p=P) for k, v in w8_t.items()}
    wv_view = {g: v.ap().rearrange("(ko p) d -> p ko d", p=P) for g, v in wv_t.items()}

    with tile.TileContext(nc) as tc, contextlib.ExitStack() as ctx:
        consts = ctx.enter_context(tc.tile_pool(name="consts", bufs=1))
        wpool = ctx.enter_context(tc.tile_pool(name="wpool", bufs=3))
        apool = ctx.enter_context(tc.tile_pool(name="apool", bufs=2))
        tmp5 = ctx.enter_context(tc.tile_pool(name="tmp5", bufs=5))
        tmp10 = ctx.enter_context(tc.tile_pool(name="tmp10", bufs=3))
        newp = ctx.enter_context(tc.tile_pool(name="newp", bufs=2))
        stat = ctx.enter_context(tc.tile_pool(name="stat", bufs=8))
        pspool = ctx.enter_context(tc.tile_pool(name="pspool", bufs=8, space="PSUM"))

        ones_f = consts.tile([P, 2], f32, name="ones_f", tag="ones_f")
        nc.vector.memset(ones_f, 1.0)
        ones8 = consts.tile([P, 2], f8, name="ones8", tag="ones8")
        nc.vector.tensor_copy(ones8, ones_f)

        eps_sb = consts.tile([P, 1], f32, name="eps_sb", tag="eps_sb")
        nc.vector.memset(eps_sb, EPS)
        shift_sb = consts.tile([P, 1], f32, name="shift_sb", tag="shift_sb")
        nc.vector.memset(shift_sb, SM_SHIFT)

        bias_col = {}
        bv_bc = {}
        if use_bias:
            for g in ("r", "z", "n"):
                for m in ("q", "k"):
                    t = consts.tile([P, DTQ], f32, name=f"b{m}{g}", tag=f"b{m}{g}")
                    nc.sync.dma_start(
                        t, b_t[g, m].ap().rearrange("(dt p) -> p dt", p=P)
                    )
                    bias_col[g, m] = t
                t = consts.tile([P, D_H], f32, name=f"bv{g}", tag=f"bv{g}")
                src = b_t[g, "v"].ap()
                nc.sync.dma_start(
                    t,
                    bass.AP(
                        tensor=src.tensor, offset=src.offset, ap=[[0, P], src.ap[0]]
                    ),
                )
                bv_bc[g] = t
        gamma_bc = beta_bc = None
        if use_gb:
            gamma_bc = consts.tile([P, D_H], f32, name="gamma_bc", tag="gamma_bc")
            beta_bc = consts.tile([P, D_H], f32, name="beta_bc", tag="beta_bc")
            for t, src_t in ((gamma_bc, gamma_t), (beta_bc, beta_t)):
                src = src_t.ap()
                nc.sync.dma_start(
                    t,
                    bass.AP(
                        tensor=src.tensor, offset=src.offset, ap=[[0, P], src.ap[0]]
                    ),
                )

        # resident fp8 V-projection weights, one tile per gate (6 MB total)
        wv_sb = {
            g: consts.tile([P, KC, D_H], f8, name=f"wv8_{g}", tag=f"wv8_{g}")
            for g in ("z", "r", "n")
        }
        # resident fp8 x-half q/k weights (3 MB total)
        w8qk_sb = {
            (g, m): consts.tile([P, KX, D_QK], f8, name=f"w8{m}_{g}", tag=f"w8{m}_{g}")
            for g in ("z", "r", "n")
            for m in ("q", "k")
        }

        assert nb % 2 == 0
        rep_ctx = tc.For_i(0, repeat, 1) if repeat > 1 else contextlib.nullcontext()
        with rep_ctx:
            for pi in range(nb // 2):
                bpair = (2 * pi, 2 * pi + 1)
                # issue the pair's first Q-weight load before its bulk
                # input loads: the HWDGE ring is FIFO, so otherwise the
                # first projection matmuls of every pair stall behind
                # the queued input tensors
                prew = {}
                w0 = wpool.tile([P, KX, P], bf16, name=f"wq0_head_{pi}", tag="wqk")
                nc.scalar.dma_start(w0, wqk_view["z", "q"][:, :, 0:P])
                prew["q", 0] = w0
                hTb_sb = {}  # bf16 h; overwritten to (h*reset) in the r gate
                x8_sb = {}
                h8_sb = {}   # fp8 h; rewritten to fp8(h*reset) in the r gate
                u_sb = {}
                for b in bpair:
                    # per-chunk DMAs so the first projection matmuls can
                    # start before the whole tensor has landed
                    hTb_sb[b] = apool.tile([P, KX, C], bf16, name=f"hTb_{b}", tag="hTb")
                    # bufs=3: the fp8 inputs are read until late in the n
                    # gate, so pair i+1 can only prefetch them into a third
                    # buffer (freed by pair i-1), not into pair i's
                    x8_sb[b] = apool.tile(
                        [P, KX, C], f8, name=f"x8_{b}", tag="x8", bufs=3
                    )
                    h8_sb[b] = apool.tile(
                        [P, KX, C], f8, name=f"h8_{b}", tag="h8", bufs=3
                    )
                    u_sb[b] = apool.tile([P, CT, D_H], bf16, name=f"u_{b}", tag="usb")
                for ko in range(KX):
                    for b in bpair:
                        hv = hTb_ap[b].rearrange("(ko p) c -> p ko c", p=P)
                        x8v = x8_ap[b].rearrange("(ko p) c -> p ko c", p=P)
                        h8v = h8_ap[b].rearrange("(ko p) c -> p ko c", p=P)
                        nc.sync.dma_start(x8_sb[b][:, ko, :], x8v[:, ko, :])
                        nc.sync.dma_start(hTb_sb[b][:, ko, :], hv[:, ko, :])
                        nc.sync.dma_start(h8_sb[b][:, ko, :], h8v[:, ko, :])
                    if pi == 0:
                        if ko == 0:
                            # z-gate fp8 q/k weights first: the very first
                            # accumulation steps are the x-half DR matmuls
                            for m in ("q", "k"):
                                nc.scalar.dma_start(
                                    w8qk_sb["z", m], w8qk_view["z", m]
                                )
                        # interleave the resident-Wv chunk loads with the
                        # input chunks so the z gate's V phase never waits
                        nc.scalar.dma_start(
                            wv_sb["z"][:, 2 * ko : 2 * ko + 2, :],
                            wv_view["z"][:, 2 * ko : 2 * ko + 2, :],
                        )
                if pi == 0:
                    for g in ("r", "n"):
                        for m in ("q", "k"):
                            nc.scalar.dma_start(w8qk_sb[g, m], w8qk_view[g, m])
                        nc.scalar.dma_start(wv_sb[g], wv_view[g])

                def v_lhsT(b, kp, e):
                    # [128, 2, 128] fp8 DoubleRow stationary slice
                    if kp < KXP:
                        return x8_sb[b][:, 2 * kp : 2 * kp + 2, e * P : (e + 1) * P]
                    kk = kp - KXP
                    return h8_sb[b][:, 2 * kk : 2 * kk + 2, e * P : (e + 1) * P]

                for gate in ("z", "r", "n"):
                    qt = {}
                    kt = {}
                    v = {}
                    # ---- Q_T / K_T projections (bf16, weight-stationary) ----
                    for b in bpair:
                        qt[b] = apool.tile([P, DTQ, C], bf16, name=f"qt_{b}", tag="qt")
                        kt[b] = apool.tile([P, DTQ, C], bf16, name=f"kt_{b}", tag="kt")
                    for m, dst in (("q", qt), ("k", kt)):
                        for dt in range(DTQ):
                            if gate == "z" and (m, dt) in prew:
                                w = prew.pop((m, dt))
                            else:
                                w = wpool.tile(
                                    [P, KX, P], bf16,
                                    name=f"w{m}{dt}_{gate}_{pi}", tag="wqk",
                                )
                                nc.scalar.dma_start(
                                    w, wqk_view[gate, m][:, :, dt * P : (dt + 1) * P]
                                )
                            # both batch elements accumulate in alternating banks:
                            # consecutive same-bank matmuls can't pipeline, so
                            # cycling banks is measurably faster on HW
                            psb = {
                                b: pspool.tile(
                                    [P, FD], f32, name=f"ps{m}{b}{dt}", tag="ps"
                                )
                                for b in bpair
                            }
                            # x-half: fp8 DoubleRow (4 chunk-pairs)
                            for kp in range(KXP):
                                w8s = w8qk_sb[gate, m][
                                    :, 2 * kp : 2 * kp + 2, dt * P : (dt + 1) * P
                                ]
                                for b in bpair:
                                    nc.tensor.matmul(
                                        psb[b],
                                        w8s,
                                        x8_sb[b][:, 2 * kp : 2 * kp + 2, :],
                                        start=(kp == 0),
                                        stop=False,
                                        perf_mode=DR,
                                    )
                            # h-half: bf16 (8 chunks) into the same PSUM group
                            for kc in range(KX):
                                for b in bpair:
                                    nc.tensor.matmul(
                                        psb[b],
                                        w[:, kc, :],
                                        hTb_sb[b][:, kc, :],
                                        start=False,
                                        stop=(kc == KX - 1),
                                    )
                            for b in bpair:
                                if use_bias:
                                    nc.vector.tensor_scalar_add(
                                        dst[b][:, dt, :],
                                        psb[b],
                                        bias_col[gate, m][:, dt : dt + 1],
                                    )
                                else:
                                    nc.vector.tensor_copy(dst[b][:, dt, :], psb[b])
                    # ---- V projection (fp8 DoubleRow, xh-stationary) ----
                    for b in bpair:
                        v[b] = apool.tile([P, ET, D_H], f8, name=f"v_{b}", tag="vv")
                    # 4-bank groups per (dh, b): each group's PSUM drains
                    # overlap the next group's matmuls, so all 8 banks are
                    # free again by the time the score matmuls need them
                    for dh in range(NDH):
                        for b in bpair:
                            pv = {
                                e: pspool.tile(
                                    [P, FD], f32, name=f"psv{b}{e}", tag="ps"
                                )
                                for e in range(ET)
                            }
                            for kp in range(KP):
                                wv_mv = wv_sb[gate][
                                    :, 2 * kp : 2 * kp + 2, dh * FD : (dh + 1) * FD
                                ]
                                for e in range(ET):
                                    nc.tensor.matmul(
                                        pv[e],
                                        v_lhsT(b, kp, e),
                                        wv_mv,
                                        start=(kp == 0),
                                        stop=(kp == KP - 1),
                                        perf_mode=DR,
                                    )
                            for e in range(ET):
                                dstv = v[b][:, e, dh * FD : (dh + 1) * FD]
                                if use_bias:
                                    nc.vector.tensor_tensor(
                                        dstv,
                                        pv[e],
                                        bv_bc[gate][:, dh * FD : (dh + 1) * FD],
                                        Alu.add,
                                    )
                                else:
                                    # ScalarE: guaranteed idle at V-phase end,
                                    # so the PSUM banks S_T waits on release
                                    # promptly (DVE may still be busy here)
                                    nc.scalar.copy(dstv, pv[e])

                    # ---- attention ----
                    # S_T+exp for BOTH batch elements are emitted before any
                    # consumer so the second element's score matmuls hide the
                    # first's exp latency on the PE stream; then the softmax
                    # denominators, then the U matmuls.
                    et = {}
                    for b in bpair:
                        et[b] = apool.tile([P, ET, C], f8, name=f"et_{b}", tag="et")
                        pse = [
                            pspool.tile([P, FD], f32, name=f"pss{b}{e}", tag="ps")
                            for e in range(ET)
                        ]
                        for dk in range(DTQ):
                            for e in range(ET):
                                nc.tensor.matmul(
                                    pse[e],
                                    kt[b][:, dk, e * P : (e + 1) * P],
                                    qt[b][:, dk, :],
                                    start=(dk == 0),
                                    stop=(dk == DTQ - 1),
                                )
                        for e in range(ET):
                            # fp8 E with a bias shift; max E ~ e^(6.9-2) stays
                            # inside e4m3's +-240 normal range
                            nc.scalar.activation(
                                et[b][:, e, :], pse[e], Act.Exp,
                                scale=SM_SCALE, bias=shift_sb,
                            )
                    den = {}
                    for b in bpair:
                        ps = pspool.tile([P, FD], f32, name=f"psd{b}", tag="ps")
                        if gate == "r":
                            # denominator as a row vector [1, c]
                            # (plain fp8: a DoubleRow ones-stationary violates
                            # the s3_lw dual-fp8 16B step restriction)
                            for e in range(ET):
                                nc.tensor.matmul(
                                    ps[0:2, :],
                                    ones8,
                                    et[b][:, e, :],
                                    start=(e == 0),
                                    stop=(e == ET - 1),
                                )
                        else:
                            # denominator as a per-partition column [c, 1]
                            for ct in range(CT):
                                for e in range(ET):
                                    nc.tensor.matmul(
                                        ps[:, 2 * ct : 2 * ct + 2],
                                        et[b][:, e, ct * P : (ct + 1) * P],
                                        ones8,
                                        start=(e == 0),
                                        stop=(e == ET - 1),
                                    )
                        den[b] = ps
                    rec = {}
                    for b in bpair:
                        if gate == "r":
                            rrow = tmp5.tile([P, C], f32, name=f"rrow{b}", tag="tmp5")
                            nc.vector.reciprocal(rrow[0:1, :], den[b][0:1, :])
                            rbc = tmp5.tile([P, C], f32, name=f"rbc{b}", tag="tmp5")
                            nc.gpsimd.partition_broadcast(rbc, rrow[0:1, :])
                            rec[b] = rbc
                        else:
                            rcol = stat.tile(
                                [P, 2 * CT], f32, name=f"rcol{b}", tag="rcol"
                            )
                            nc.vector.reciprocal(rcol, den[b][:, 0 : 2 * CT])
                            rec[b] = rcol
                    for b in bpair:
                        if gate == "r":
                            # U_rT[d,c]/s -> sigmoid -> h tiles *= reset (in place)
                            for dt0 in range(0, DTH, 2):
                                dts = (dt0, dt0 + 1)
                                psr = {
                                    d: pspool.tile(
                                        [P, FD], f32, name=f"psu{b}{d}", tag="ps"
                                    )
                                    for d in dts
                                }
                                for ep in range(EP):
                                    for d in dts:
                                        nc.tensor.matmul(
                                            psr[d],
                                            v[b][
                                                :, 2 * ep : 2 * ep + 2,
                                                d * P : (d + 1) * P,
                                            ],
                                            et[b][:, 2 * ep : 2 * ep + 2, :],
                                            start=(ep == 0),
                                            stop=(ep == EP - 1),
                                            perf_mode=DR,
                                        )
                                for d in dts:
                                    # 1/s applied after the matmul so the U
                                    # matmuls never wait on it
                                    sg = tmp5.tile(
                                        [P, C], f32, name=f"sg{b}{d}", tag="tmp5"
                                    )
                                    nc.vector.tensor_tensor(sg, psr[d], rec[b], Alu.mult)
                                    sgb = tmp5.tile(
                                        [P, C], bf16, name=f"sgb{b}{d}", tag="sgb"
                                    )
                                    nc.scalar.activation(sgb, sg, Act.Sigmoid)
                                    nc.vector.tensor_tensor(
                                        hTb_sb[b][:, d, :], hTb_sb[b][:, d, :],
                                        sgb, Alu.mult,
                                    )
                                    nc.vector.tensor_copy(
                                        h8_sb[b][:, d, :], hTb_sb[b][:, d, :]
                                    )
                        elif gate == "z":
                            for ct in range(CT):
                                pu = {
                                    dh: pspool.tile(
                                        [P, FD], f32, name=f"psu{b}{ct}{dh}", tag="ps"
                                    )
                                    for dh in range(NDH)
                                }
                                for ep in range(EP):
                                    for dh in range(NDH):
                                        nc.tensor.matmul(
                                            pu[dh],
                                            et[b][
                                                :, 2 * ep : 2 * ep + 2,
                                                ct * P : (ct + 1) * P,
                                            ],
                                            v[b][
                                                :, 2 * ep : 2 * ep + 2,
                                                dh * FD : (dh + 1) * FD,
                                            ],
                                            start=(ep == 0),
                                            stop=(ep == EP - 1),
                                            perf_mode=DR,
                                        )
                                for dh in range(NDH):
                                    # update gate straight into the bf16 SBUF
                                    # scratch the n gate reads back
                                    nc.scalar.activation(
                                        u_sb[b][:, ct, dh * FD : (dh + 1) * FD],
                                        pu[dh], Act.Sigmoid,
                                        scale=rec[b][:, 2 * ct : 2 * ct + 1],
                                    )
                        else:  # gate == "n": fused gating + LayerNorm
                            for ct in range(CT):
                                hN_ct = tmp10.tile(
                                    [P, D_H], f32, name=f"hN{b}{ct}", tag="tmp10"
                                )
                                nc.sync.dma_start(
                                    hN_ct, hN_ap[b, ct * P : (ct + 1) * P, :]
                                )
                                new_t = newp.tile(
                                    [P, D_H], f32, name=f"new{b}{ct}", tag="new"
                                )
                                pu = {
                                    dh: pspool.tile(
                                        [P, FD], f32, name=f"psu{b}{ct}{dh}", tag="ps"
                                    )
                                    for dh in range(NDH)
                                }
                                for ep in range(EP):
                                    for dh in range(NDH):
                                        nc.tensor.matmul(
                                            pu[dh],
                                            et[b][
                                                :, 2 * ep : 2 * ep + 2,
                                                ct * P : (ct + 1) * P,
                                            ],
                                            v[b][
                                                :, 2 * ep : 2 * ep + 2,
                                                dh * FD : (dh + 1) * FD,
                                            ],
                                            start=(ep == 0),
                                            stop=(ep == EP - 1),
                                            perf_mode=DR,
                                        )
                                for dh in range(NDH):
                                    tt = tmp5.tile(
                                        [P, FD], f32, name=f"tt{b}{ct}{dh}", tag="tmp5"
                                    )
                                    nc.scalar.activation(
                                        tt, pu[dh], Act.Tanh,
                                        scale=rec[b][:, 2 * ct : 2 * ct + 1],
                                    )
                                    hsl = hN_ct[:, dh * FD : (dh + 1) * FD]
                                    usl = u_sb[b][:, ct, dh * FD : (dh + 1) * FD]
                                    nsl = new_t[:, dh * FD : (dh + 1) * FD]
                                    # new = h + u*(tanh(n) - h)
                                    nc.vector.tensor_tensor(tt, tt, hsl, Alu.subtract)
                                    nc.vector.tensor_tensor(tt, tt, usl, Alu.mult)
                                    nc.vector.tensor_tensor(nsl, tt, hsl, Alu.add)
                                # ---- LayerNorm over d per token row ----
                                stats = stat.tile(
                                    [P, 2, 6], f32, name=f"st{b}{ct}", tag="st"
                                )
                                for half in range(2):
                                    nc.vector.bn_stats(
                                        stats[:, half, :],
                                        new_t[:, half * FD : (half + 1) * FD],
                                    )
                                mv = stat.tile([P, 2], f32, name=f"mv{b}{ct}", tag="mv")
                                nc.vector.bn_aggr(mv, stats)
                                rstd = stat.tile(
                                    [P, 1], f32, name=f"rs{b}{ct}", tag="rs"
                                )
                                nc.scalar.activation(
                                    rstd, mv[:, 1:2], Act.Sqrt, bias=eps_sb
                                )
                                nc.vector.reciprocal(rstd, rstd)
                                nc.vector.tensor_scalar(
                                    new_t,
                                    new_t,
                                    mv[:, 0:1],
                                    rstd,
                                    op0=Alu.subtract,
                                    op1=Alu.mult,
                                )
                                if use_gb:
                                    nc.vector.tensor_tensor(
                                        new_t, new_t, gamma_bc, Alu.mult
                                    )
                                    nc.vector.tensor_tensor(
                                        new_t, new_t, beta_bc, Alu.add
                                    )
                                # SWDGE via the idle GPSIMD engine: keeps
                                # the SP HWDGE ring free for the next pair's
                                # input prefetch (stores have no consumer)
                                nc.gpsimd.dma_start(
                                    out_ap[b, ct * P : (ct + 1) * P, :], new_t
                                )

    nc.compile()
    return nc


def _get_nc(nb, use_bias, use_gb):
    key = (nb, use_bias, use_gb)
    if key not in _CACHE:
        _CACHE[key] = _build(nb, use_bias, use_gb)
    return _CACHE[key]


def _make_in_maps(inputs, nb=NB, n_cores=N_CORES):
    import ml_dtypes

    BF16 = ml_dtypes.bfloat16
    F8 = ml_dtypes.float8_e4m3

    def q8(a):
        return np.clip(a, -240.0, 240.0).astype(F8)

    x = np.ascontiguousarray(np.asarray(inputs["x"], dtype=np.float32))
    h = np.ascontiguousarray(np.asarray(inputs["h"], dtype=np.float32))
    shared = {}
    for g in ("r", "z", "n"):
        for m in ("q", "k"):
            w = np.asarray(inputs[f"W{m}_{g}"], dtype=np.float32)
            shared[f"W{m}_{g}"] = np.ascontiguousarray(w[D_IN:].astype(BF16))
            shared[f"W{m}8_{g}"] = np.ascontiguousarray(q8(w[:D_IN]))
        shared[f"Wv8_{g}"] = np.ascontiguousarray(
            q8(np.asarray(inputs[f"Wv_{g}"], dtype=np.float32))
        )
        for nm in ("bq", "bk", "bv"):
            shared[f"{nm}_{g}"] = np.ascontiguousarray(
                np.asarray(inputs[f"{nm}_{g}"], dtype=np.float32)
            )
    shared["gamma"] = np.ascontiguousarray(np.asarray(inputs["gamma"], np.float32))
    shared["beta"] = np.ascontiguousarray(np.asarray(inputs["beta"], np.float32))

    in_maps = []
    for ci in range(n_cores):
        sl = slice(ci * nb, (ci + 1) * nb)
        xs = x[sl]
        hs = h[sl]
        xT = xs.transpose(0, 2, 1)
        hT = hs.transpose(0, 2, 1)
        m = dict(shared)
        m["hTb"] = np.ascontiguousarray(hT.astype(BF16))
        m["x8"] = np.ascontiguousarray(q8(xT))
        m["h8"] = np.ascontiguousarray(q8(hT))
        m["hN"] = hs
        in_maps.append(m)
    return in_maps


def _flags(inputs):
    use_bias = any(
        np.any(np.asarray(inputs[f"{nm}_{g}"]))
        for g in ("r", "z", "n")
        for nm in ("bq", "bk", "bv")
    )
    gamma = np.asarray(inputs["gamma"])
    beta = np.asarray(inputs["beta"])
    use_gb = (not np.allclose(gamma, 1.0)) or bool(np.any(beta))
    return bool(use_bias), bool(use_gb)


def kernel(**inputs):
    from concourse import bass_utils

    use_bias, use_gb = _flags(inputs)
    nc = _get_nc(NB, use_bias, use_gb)
    in_maps = _make_in_maps(inputs)
    res = bass_utils.run_bass_kernel_spmd(nc, in_maps, core_ids=list(range(N_CORES)))
    out = np.concatenate([r["out"] for r in res.results], axis=0)
    return np.ascontiguousarray(out.astype(np.float32))


# revision 43
# speedup vs baseline: 1.1390x; 1.1390x over previous
"""Trainium2 Bass kernel for nn_ChannelAttGatedGRUCell.

Reference computation (per batch element b):
    xh = concat([x, h], -1)                                  # (C, 2048)
    r = attn(xh; Wq_r, Wk_r, Wv_r); z = attn(xh; ...z)       # (C, 1024)
    reset = sigmoid(r); update = sigmoid(z)
    xhr = concat([x, h*reset], -1)
    n = attn(xhr; ...n)
    new = (1-update)*h + update*tanh(n)
    out = LayerNorm(new) * gamma + beta

Sharding: data-parallel over batch B=64 across 8 cores (8 per core);
weights replicated.  The host pre-transposes x/h to feature-major and
pre-casts each tensor to the precision its consumers need.

Mixed precision (absmax-rel budget 2e-2; measured 1.62e-2 on HW, CPU
simulation of the exact quantization dataflow predicted 1.78e-2):
  - fp8 e4m3 + DoubleRow perf mode (2 contraction chunks per
    instruction via [128, 2, N] operands, 2x ALU rate) for: the V
    projections, the attn@V contractions, and the x-half of the Q/K
    projections (x is quantized once on the host, shared by all).
  - bf16 (1 cycle/row) for the h-half of the Q/K projections and the
    score matmuls: CPU simulation shows the score path dominates the
    error budget, so it keeps ~8 mantissa bits.  The h-half fp8+bf16
    accumulate into one PSUM group (measured: no mode-switch penalty).
  - exp() is emitted straight from PSUM with a -2.0 bias shift so the
    fp8 attention weights stay inside TRN e4m3's +-240 normal range;
    the softmax denominator sums the *quantized* weights, so
    normalization is exact w.r.t. the fp8 rounding.
  - Final gating + LayerNorm in fp32 (update gate held in SBUF bf16).

On-device dataflow per batch element (layouts avoid all transposes):
    Q_T[d,c]  = Wq8[k,d].T @ x8[k,c] + Wq[k,d].T @ hT[k,c]
    K_T[d,c]  = (same split as Q_T)
    V[e,d]    = x8/h8[k,e].T @ Wv8[k,d]    (fp8 DR; Wv8 resident, 6MB)
    S_T[e,c]  = K_T[d,e].T @ Q_T[d,c]      (bf16)
    E_T[e,c]  = exp(S_T/sqrt(dqk) - 2)     (ScalarE, PSUM -> fp8 SBUF)
    s[c]      = ones.T @ E_T               (softmax denom via matmul)
    U[c,d]    = E_T[e,c].T @ V[e,d]        (fp8 DR; z/n gates)
    U_rT[d,c] = V[e,d].T @ E_T[e,c]        (fp8 DR; r gate)
    1/s folds into the ScalarE sigmoid/tanh `scale` operand (z/n) or
    into one DVE rescale of U_rT (r gate).

Gate order is (z, r, n): after z no longer needs h, the r gate's reset
multiply overwrites the bf16 h tile in place with (h*reset) and then
re-quantizes it over the fp8 h tile, which the n gate reads as the
second half of xhr.  The z update gate stays in SBUF as bf16.  PSUM
banks alternate per batch element / e-tile inside every accumulation
phase (consecutive same-bank matmuls cannot pipeline), and the V
phase runs in 4-bank groups so its drains overlap the next group.
"""

import numpy as np

B, C, D_IN, D_H, D_QK = 64, 512, 1024, 1024, 512
DIN2 = D_IN + D_H
N_CORES = 8
NB = B // N_CORES          # batch elements per core
EPS = 1e-5
P = 128
KC = DIN2 // P             # 16 contraction chunks over concat features
KX = D_IN // P             # 8  (x-half chunks; h-half is chunks KX..KC-1)
KP = KC // 2               # 8  DoubleRow chunk-pairs over concat features
KXP = KX // 2              # 4  DoubleRow pairs in the x half
CT = C // P                # 4  c-tiles (tokens)
ET = CT                    # 4  e-tiles (tokens, attended-over axis)
EP = ET // 2               # 2  DoubleRow e-tile pairs
DTQ = D_QK // P            # 4  d-tiles of q/k head dim
DTH = D_H // P             # 8  d-tiles of value dim
FD = 512                   # matmul moving free dim (fp32 PSUM bank)
NDH = D_H // FD            # 2  d-halves of value dim
SM_SCALE = float(1.0 / np.sqrt(D_QK))
SM_SHIFT = -2.0            # exp bias shift keeping fp8 E in normal range

_CACHE = {}


def _build(nb, use_bias, use_gb, repeat=1):
    """Build + compile the per-core Bass program for nb batch elements.

    repeat>1 wraps the body in a hardware For_i loop that recomputes the
    same outputs; used only to measure per-iteration HW time."""
    import contextlib

    import concourse.bacc as bacc
    import concourse.bass as bass
    import concourse.mybir as mybir
    import concourse.tile as tile

    f32 = mybir.dt.float32
    bf16 = mybir.dt.bfloat16
    f8 = mybir.dt.float8e4
    Alu = mybir.AluOpType
    Act = mybir.ActivationFunctionType
    DR = mybir.MatmulPerfMode.DoubleRow

    nc = bacc.Bacc("TRN2", target_bir_lowering=False, debug=False)

    # ---- DRAM I/O ----
    hTb_t = nc.dram_tensor("hTb", [nb, D_H, C], bf16, kind="ExternalInput")
    x8_t = nc.dram_tensor("x8", [nb, D_IN, C], f8, kind="ExternalInput")
    h8_t = nc.dram_tensor("h8", [nb, D_H, C], f8, kind="ExternalInput")
    hN_t = nc.dram_tensor("hN", [nb, C, D_H], f32, kind="ExternalInput")
    w_t = {}
    w8_t = {}
    wv_t = {}
    b_t = {}
    for g in ("r", "z", "n"):
        # h-half of the q/k weights stays bf16; the x-half is fp8 and runs
        # in DoubleRow mode (the x input is quantized once, shared with the
        # V projections)
        w_t[g, "q"] = nc.dram_tensor(f"Wq_{g}", [D_H, D_QK], bf16, kind="ExternalInput")
        w_t[g, "k"] = nc.dram_tensor(f"Wk_{g}", [D_H, D_QK], bf16, kind="ExternalInput")
        w8_t[g, "q"] = nc.dram_tensor(f"Wq8_{g}", [D_IN, D_QK], f8, kind="ExternalInput")
        w8_t[g, "k"] = nc.dram_tensor(f"Wk8_{g}", [D_IN, D_QK], f8, kind="ExternalInput")
        wv_t[g] = nc.dram_tensor(f"Wv8_{g}", [DIN2, D_H], f8, kind="ExternalInput")
        b_t[g, "q"] = nc.dram_tensor(f"bq_{g}", [D_QK], f32, kind="ExternalInput")
        b_t[g, "k"] = nc.dram_tensor(f"bk_{g}", [D_QK], f32, kind="ExternalInput")
        b_t[g, "v"] = nc.dram_tensor(f"bv_{g}", [D_H], f32, kind="ExternalInput")
    gamma_t = nc.dram_tensor("gamma", [D_H], f32, kind="ExternalInput")
    beta_t = nc.dram_tensor("beta", [D_H], f32, kind="ExternalInput")
    out_t = nc.dram_tensor("out", [nb, C, D_H], f32, kind="ExternalOutput")

    hTb_ap = hTb_t.ap()
    x8_ap = x8_t.ap()
    h8_ap = h8_t.ap()
    hN_ap = hN_t.ap()
    out_ap = out_t.ap()
    # feature-major [128, ko, free] views of weights
    wqk_view = {k: v.ap().rearrange("(ko p) d -> p ko d", p=P) for k, v in w_t.items()}
    w8qk_view = {k: v.ap().rearrange("(ko p) d -> p ko d", p=P) for k, v in w8_t.items()}
    wv_view = {g: v.ap().rearrange("(ko p) d -> p ko d", p=P) for g, v in wv_t.items()}

    with tile.TileContext(nc) as tc, contextlib.ExitStack() as ctx:
        consts = ctx.enter_context(tc.tile_pool(name="consts", bufs=1))
        wpool = ctx.enter_context(tc.tile_pool(name="wpool", bufs=3))
        apool = ctx.enter_context(tc.tile_pool(name="apool", bufs=2))
        tmp5 = ctx.enter_context(tc.tile_pool(name="tmp5", bufs=5))
        tmp10 = ctx.enter_context(tc.tile_pool(name="tmp10", bufs=3))
        newp = ctx.enter_context(tc.tile_pool(name="newp", bufs=2))
        stat = ctx.enter_context(tc.tile_pool(name="stat", bufs=8))
        pspool = ctx.enter_context(tc.tile_pool(name="pspool", bufs=8, space="PSUM"))

        ones_f = consts.tile([P, 2], f32, name="ones_f", tag="ones_f")
        nc.vector.memset(ones_f, 1.0)
        ones8 = consts.tile([P, 2], f8, name="ones8", tag="ones8")
        nc.vector.tensor_copy(ones8, ones_f)

        eps_sb = consts.tile([P, 1], f32, name="eps_sb", tag="eps_sb")
        nc.vector.memset(eps_sb, EPS)
        shift_sb = consts.tile([P, 1], f32, name="shift_sb", tag="shift_sb")
        nc.vector.memset(shift_sb, SM_SHIFT)

        bias_col = {}
        bv_bc = {}
        if use_bias:
            for g in ("r", "z", "n"):
                for m in ("q", "k"):
                    t = consts.tile([P, DTQ], f32, name=f"b{m}{g}", tag=f"b{m}{g}")
                    nc.sync.dma_start(
                        t, b_t[g, m].ap().rearrange("(dt p) -> p dt", p=P)
                    )
                    bias_col[g, m] = t
                t = consts.tile([P, D_H], f32, name=f"bv{g}", tag=f"bv{g}")
                src = b_t[g, "v"].ap()
                nc.sync.dma_start(
                    t,
                    bass.AP(
                        tensor=src.tensor, offset=src.offset, ap=[[0, P], src.ap[0]]
                    ),
                )
                bv_bc[g] = t
        gamma_bc = beta_bc = None
        if use_gb:
            gamma_bc = consts.tile([P, D_H], f32, name="gamma_bc", tag="gamma_bc")
            beta_bc = consts.tile([P, D_H], f32, name="beta_bc", tag="beta_bc")
            for t, src_t in ((gamma_bc, gamma_t), (beta_bc, beta_t)):
                src = src_t.ap()
                nc.sync.dma_start(
                    t,
                    bass.AP(
                        tensor=src.tensor, offset=src.offset, ap=[[0, P], src.ap[0]]
                    ),
                )

        # resident fp8 V-projection weights, one tile per gate (6 MB total)
        wv_sb = {
            g: consts.tile([P, KC, D_H], f8, name=f"wv8_{g}", tag=f"wv8_{g}")
            for g in ("z", "r", "n")
        }
        # resident fp8 x-half q/k weights (3 MB total)
        w8qk_sb = {
            (g, m): consts.tile([P, KX, D_QK], f8, name=f"w8{m}_{g}", tag=f"w8{m}_{g}")
            for g in ("z", "r", "n")
            for m in ("q", "k")
        }

        assert nb % 2 == 0
        rep_ctx = tc.For_i(0, repeat, 1) if repeat > 1 else contextlib.nullcontext()
        with rep_ctx:
            for pi in range(nb // 2):
                bpair = (2 * pi, 2 * pi + 1)
                # issue the pair's first Q-weight load before its bulk
                # input loads: the HWDGE ring is FIFO, so otherwise the
                # first projection matmuls of every pair stall behind
                # the queued input tensors
                prew = {}
                w0 = wpool.tile([P, KX, P], bf16, name=f"wq0_head_{pi}", tag="wqk")
                nc.scalar.dma_start(w0, wqk_view["z", "q"][:, :, 0:P])
                prew["q", 0] = w0
                w1 = wpool.tile([P, KX, P], bf16, name=f"wq1_head_{pi}", tag="wqk")
                nc.scalar.dma_start(w1, wqk_view["z", "q"][:, :, P : 2 * P])
                prew["q", 1] = w1
                hTb_sb = {}  # bf16 h; overwritten to (h*reset) in the r gate
                x8_sb = {}
                h8_sb = {}   # fp8 h; rewritten to fp8(h*reset) in the r gate
                u_sb = {}
                for b in bpair:
                    # per-chunk DMAs so the first projection matmuls can
                    # start before the whole tensor has landed
                    hTb_sb[b] = apool.tile([P, KX, C], bf16, name=f"hTb_{b}", tag="hTb")
                    # bufs=3: the fp8 inputs are read until late in the n
                    # gate, so pair i+1 can only prefetch them into a third
                    # buffer (freed by pair i-1), not into pair i's
                    x8_sb[b] = apool.tile(
                        [P, KX, C], f8, name=f"x8_{b}", tag="x8", bufs=3
                    )
                    h8_sb[b] = apool.tile(
                        [P, KX, C], f8, name=f"h8_{b}", tag="h8", bufs=3
                    )
                    u_sb[b] = apool.tile([P, CT, D_H], bf16, name=f"u_{b}", tag="usb")
                for ko in range(KX):
                    for b in bpair:
                        hv = hTb_ap[b].rearrange("(ko p) c -> p ko c", p=P)
                        x8v = x8_ap[b].rearrange("(ko p) c -> p ko c", p=P)
                        h8v = h8_ap[b].rearrange("(ko p) c -> p ko c", p=P)
                        nc.sync.dma_start(x8_sb[b][:, ko, :], x8v[:, ko, :])
                        nc.sync.dma_start(hTb_sb[b][:, ko, :], hv[:, ko, :])
                        nc.sync.dma_start(h8_sb[b][:, ko, :], h8v[:, ko, :])
                    if pi == 0:
                        if ko == 0:
                            # z-gate fp8 q/k weights first: the very first
                            # accumulation steps are the x-half DR matmuls
                            for m in ("q", "k"):
                                nc.scalar.dma_start(
                                    w8qk_sb["z", m], w8qk_view["z", m]
                                )
                        # interleave the resident-Wv chunk loads with the
                        # input chunks so the z gate's V phase never waits
                        nc.scalar.dma_start(
                            wv_sb["z"][:, 2 * ko : 2 * ko + 2, :],
                            wv_view["z"][:, 2 * ko : 2 * ko + 2, :],
                        )
                if pi == 0:
                    for g in ("r", "n"):
                        for m in ("q", "k"):
                            nc.scalar.dma_start(w8qk_sb[g, m], w8qk_view[g, m])
                        nc.scalar.dma_start(wv_sb[g], wv_view[g])

                def v_lhsT(b, kp, e):
                    # [128, 2, 128] fp8 DoubleRow stationary slice
                    if kp < KXP:
                        return x8_sb[b][:, 2 * kp : 2 * kp + 2, e * P : (e + 1) * P]
                    kk = kp - KXP
                    return h8_sb[b][:, 2 * kk : 2 * kk + 2, e * P : (e + 1) * P]

                for gate in ("z", "r", "n"):
                    qt = {}
                    kt = {}
                    v = {}
                    # ---- Q_T / K_T projections (bf16, weight-stationary) ----
                    for b in bpair:
                        qt[b] = apool.tile([P, DTQ, C], bf16, name=f"qt_{b}", tag="qt")
                        kt[b] = apool.tile([P, DTQ, C], bf16, name=f"kt_{b}", tag="kt")
                    for m, dst in (("q", qt), ("k", kt)):
                        for dt in range(DTQ):
                            if gate == "z" and (m, dt) in prew:
                                w = prew.pop((m, dt))
                            else:
                                w = wpool.tile(
                                    [P, KX, P], bf16,
                                    name=f"w{m}{dt}_{gate}_{pi}", tag="wqk",
                                )
                                nc.scalar.dma_start(
                                    w, wqk_view[gate, m][:, :, dt * P : (dt + 1) * P]
                                )
                            # both batch elements accumulate in alternating banks:
                            # consecutive same-bank matmuls can't pipeline, so
                            # cycling banks is measurably faster on HW
                            psb = {
                                b: pspool.tile(
                                    [P, FD], f32, name=f"ps{m}{b}{dt}", tag="ps"
                                )
                                for b in bpair
                            }
                            # x-half: fp8 DoubleRow (4 chunk-pairs)
                            for kp in range(KXP):
                                w8s = w8qk_sb[gate, m][
                                    :, 2 * kp : 2 * kp + 2, dt * P : (dt + 1) * P
                                ]
                                for b in bpair:
                                    nc.tensor.matmul(
                                        psb[b],
                                        w8s,
                                        x8_sb[b][:, 2 * kp : 2 * kp + 2, :],
                                        start=(kp == 0),
                                        stop=False,
                                        perf_mode=DR,
                                    )
                            # h-half: bf16 (8 chunks) into the same PSUM group
                            for kc in range(KX):
                                for b in bpair:
                                    nc.tensor.matmul(
                                        psb[b],
                                        w[:, kc, :],
                                        hTb_sb[b][:, kc, :],
                                        start=False,
                                        stop=(kc == KX - 1),
                                    )
                            for b in bpair:
                                if use_bias:
                                    nc.vector.tensor_scalar_add(
                                        dst[b][:, dt, :],
                                        psb[b],
                                        bias_col[gate, m][:, dt : dt + 1],
                                    )
                                else:
                                    nc.vector.tensor_copy(dst[b][:, dt, :], psb[b])
                    # ---- V projection (fp8 DoubleRow, xh-stationary) ----
                    for b in bpair:
                        v[b] = apool.tile([P, ET, D_H], f8, name=f"v_{b}", tag="vv")
                    # 4-bank groups per (dh, b): each group's PSUM drains
                    # overlap the next group's matmuls, so all 8 banks are
                    # free again by the time the score matmuls need them
                    for dh in range(NDH):
                        for b in bpair:
                            pv = {
                                e: pspool.tile(
                                    [P, FD], f32, name=f"psv{b}{e}", tag="ps"
                                )
                                for e in range(ET)
                            }
                            for kp in range(KP):
                                wv_mv = wv_sb[gate][
                                    :, 2 * kp : 2 * kp + 2, dh * FD : (dh + 1) * FD
                                ]
                                for e in range(ET):
                                    nc.tensor.matmul(
                                        pv[e],
                                        v_lhsT(b, kp, e),
                                        wv_mv,
                                        start=(kp == 0),
                                        stop=(kp == KP - 1),
                                        perf_mode=DR,
                                    )
                            for e in range(ET):
                                dstv = v[b][:, e, dh * FD : (dh + 1) * FD]
                                if use_bias:
                                    nc.vector.tensor_tensor(
                                        dstv,
                                        pv[e],
                                        bv_bc[gate][:, dh * FD : (dh + 1) * FD],
                                        Alu.add,
                                    )
                                else:
                                    # ScalarE: guaranteed idle at V-phase end,
                                    # so the PSUM banks S_T waits on release
                                    # promptly (DVE may still be busy here)
                                    nc.scalar.copy(dstv, pv[e])

                    # ---- attention ----
                    # S_T+exp for BOTH batch elements are emitted before any
                    # consumer so the second element's score matmuls hide the
                    # first's exp latency on the PE stream; then the softmax
                    # denominators, then the U matmuls.
                    et = {}
                    for b in bpair:
                        et[b] = apool.tile([P, ET, C], f8, name=f"et_{b}", tag="et")
                        pse = [
                            pspool.tile([P, FD], f32, name=f"pss{b}{e}", tag="ps")
                            for e in range(ET)
                        ]
                        for dk in range(DTQ):
                            for e in range(ET):
                                nc.tensor.matmul(
                                    pse[e],
                                    kt[b][:, dk, e * P : (e + 1) * P],
                                    qt[b][:, dk, :],
                                    start=(dk == 0),
                                    stop=(dk == DTQ - 1),
                                )
                        for e in range(ET):
                            # fp8 E with a bias shift; max E ~ e^(6.9-2) stays
                            # inside e4m3's +-240 normal range
                            nc.scalar.activation(
                                et[b][:, e, :], pse[e], Act.Exp,
                                scale=SM_SCALE, bias=shift_sb,
                            )
                    den = {}
                    for b in bpair:
                        ps = pspool.tile([P, FD], f32, name=f"psd{b}", tag="ps")
                        if gate == "r":
                            # denominator as a row vector [1, c]
                            # (plain fp8: a DoubleRow ones-stationary violates
                            # the s3_lw dual-fp8 16B step restriction)
                            for e in range(ET):
                                nc.tensor.matmul(
                                    ps[0:2, :],
                                    ones8,
                                    et[b][:, e, :],
                                    start=(e == 0),
                                    stop=(e == ET - 1),
                                )
                        else:
                            # denominator as a per-partition column [c, 1]
                            for ct in range(CT):
                                for e in range(ET):
                                    nc.tensor.matmul(
                                        ps[:, 2 * ct : 2 * ct + 2],
                                        et[b][:, e, ct * P : (ct + 1) * P],
                                        ones8,
                                        start=(e == 0),
                                        stop=(e == ET - 1),
                                    )
                        den[b] = ps
                    rec = {}
                    for b in bpair:
                        if gate == "r":
                            rrow = tmp5.tile([P, C], f32, name=f"rrow{b}", tag="tmp5")
                            nc.vector.reciprocal(rrow[0:1, :], den[b][0:1, :])
                            rbc = tmp5.tile([P, C], f32, name=f"rbc{b}", tag="tmp5")
                            nc.gpsimd.partition_broadcast(rbc, rrow[0:1, :])
                            rec[b] = rbc
                        else:
                            rcol = stat.tile(
                                [P, 2 * CT], f32, name=f"rcol{b}", tag="rcol"
                            )
                            nc.vector.reciprocal(rcol, den[b][:, 0 : 2 * CT])
                            rec[b] = rcol
                    for b in bpair:
                        if gate == "r":
                            # U_rT[d,c]/s -> sigmoid -> h tiles *= reset (in place)
                            for dt0 in range(0, DTH, 2):
                                dts = (dt0, dt0 + 1)
                                psr = {
                                    d: pspool.tile(
                                        [P, FD], f32, name=f"psu{b}{d}", tag="ps"
                                    )
                                    for d in dts
                                }
                                for ep in range(EP):
                                    for d in dts:
                                        nc.tensor.matmul(
                                            psr[d],
                                            v[b][
                                                :, 2 * ep : 2 * ep + 2,
                                                d * P : (d + 1) * P,
                                            ],
                                            et[b][:, 2 * ep : 2 * ep + 2, :],
                                            start=(ep == 0),
                                            stop=(ep == EP - 1),
                                            perf_mode=DR,
                                        )
                                for d in dts:
                                    # 1/s applied after the matmul so the U
                                    # matmuls never wait on it
                                    sg = tmp5.tile(
                                        [P, C], f32, name=f"sg{b}{d}", tag="tmp5"
                                    )
                                    nc.vector.tensor_tensor(sg, psr[d], rec[b], Alu.mult)
                                    sgb = tmp5.tile(
                                        [P, C], bf16, name=f"sgb{b}{d}", tag="sgb"
                                    )
                                    nc.scalar.activation(sgb, sg, Act.Sigmoid)
                                    nc.vector.tensor_tensor(
                                        hTb_sb[b][:, d, :], hTb_sb[b][:, d, :],
                                        sgb, Alu.mult,
                                    )
                                    nc.vector.tensor_copy(
                                        h8_sb[b][:, d, :], hTb_sb[b][:, d, :]
                                    )
                        elif gate == "z":
                            for ct in range(CT):
                                pu = {
                                    dh: pspool.tile(
                                        [P, FD], f32, name=f"psu{b}{ct}{dh}", tag="ps"
                                    )
                                    for dh in range(NDH)
                                }
                                for ep in range(EP):
                                    for dh in range(NDH):
                                        nc.tensor.matmul(
                                            pu[dh],
                                            et[b][
                                                :, 2 * ep : 2 * ep + 2,
                                                ct * P : (ct + 1) * P,
                                            ],
                                            v[b][
                                                :, 2 * ep : 2 * ep + 2,
                                                dh * FD : (dh + 1) * FD,
                                            ],
                                            start=(ep == 0),
                                            stop=(ep == EP - 1),
                                            perf_mode=DR,
                                        )
                                for dh in range(NDH):
                                    # update gate straight into the bf16 SBUF
                                    # scratch the n gate reads back
                                    nc.scalar.activation(
                                        u_sb[b][:, ct, dh * FD : (dh + 1) * FD],
                                        pu[dh], Act.Sigmoid,
                                        scale=rec[b][:, 2 * ct : 2 * ct + 1],
                                    )
                        else:  # gate == "n": fused gating + LayerNorm
                            # prefetch all hN chunks up front: the gating
                            # chain must never wait on a just-issued DMA
                            hN_tiles = []
                            for ct in range(CT):
                                hN_ct = tmp10.tile(
                                    [P, D_H], f32, name=f"hN{b}{ct}", tag="tmp10"
                                )
                                nc.sync.dma_start(
                                    hN_ct, hN_ap[b, ct * P : (ct + 1) * P, :]
                                )
                                hN_tiles.append(hN_ct)
                            for ct in range(CT):
                                hN_ct = hN_tiles[ct]
                                new_t = newp.tile(
                                    [P, D_H], f32, name=f"new{b}{ct}", tag="new"
                                )
                                pu = {
                                    dh: pspool.tile(
                                        [P, FD], f32, name=f"psu{b}{ct}{dh}", tag="ps"
                                    )
                                    for dh in range(NDH)
                                }
                                for ep in range(EP):
                                    for dh in range(NDH):
                                        nc.tensor.matmul(
                                            pu[dh],
                                            et[b][
                                                :, 2 * ep : 2 * ep + 2,
                                                ct * P : (ct + 1) * P,
                                            ],
                                            v[b][
                                                :, 2 * ep : 2 * ep + 2,
                                                dh * FD : (dh + 1) * FD,
                                            ],
                                            start=(ep == 0),
                                            stop=(ep == EP - 1),
                                            perf_mode=DR,
                                        )
                                for dh in range(NDH):
                                    tt = tmp5.tile(
                                        [P, FD], f32, name=f"tt{b}{ct}{dh}", tag="tmp5"
                                    )
                                    nc.scalar.activation(
                                        tt, pu[dh], Act.Tanh,
                                        scale=rec[b][:, 2 * ct : 2 * ct + 1],
                                    )
                                    hsl = hN_ct[:, dh * FD : (dh + 1) * FD]
                                    usl = u_sb[b][:, ct, dh * FD : (dh + 1) * FD]
                                    nsl = new_t[:, dh * FD : (dh + 1) * FD]
                                    # new = h + u*(tanh(n) - h)
                                    nc.vector.tensor_tensor(tt, tt, hsl, Alu.subtract)
                                    nc.vector.tensor_tensor(tt, tt, usl, Alu.mult)
                                    nc.vector.tensor_tensor(nsl, tt, hsl, Alu.add)
                                # ---- LayerNorm over d per token row ----
                                stats = stat.tile(
                                    [P, 2, 6], f32, name=f"st{b}{ct}", tag="st"
                                )
                                for half in range(2):
                                    nc.vector.bn_stats(
                                        stats[:, half, :],
                                        new_t[:, half * FD : (half + 1) * FD],
                                    )
                                mv = stat.tile([P, 2], f32, name=f"mv{b}{ct}", tag="mv")
                                nc.vector.bn_aggr(mv, stats)
                                rstd = stat.tile(
                                    [P, 1], f32, name=f"rs{b}{ct}", tag="rs"
                                )
                                nc.scalar.activation(
                                    rstd, mv[:, 1:2], Act.Sqrt, bias=eps_sb
                                )
                                nc.vector.reciprocal(rstd, rstd)
                                nc.vector.tensor_scalar(
                                    new_t,
                                    new_t,
                                    mv[:, 0:1],
                                    rstd,
                                    op0=Alu.subtract,
                                    op1=Alu.mult,
                                )
                                if use_gb:
                                    nc.vector.tensor_tensor(
                                        new_t, new_t, gamma_bc, Alu.mult
                                    )
                                    nc.vector.tensor_tensor(
                                        new_t, new_t, beta_bc, Alu.add
                                    )
                                nc.sync.dma_start(
                                    out_ap[b, ct * P : (ct + 1) * P, :], new_t
                                )

    nc.compile()
    return nc


def _get_nc(nb, use_bias, use_gb):
    key = (nb, use_bias, use_gb)
    if key not in _CACHE:
        _CACHE[key] = _build(nb, use_bias, use_gb)
    return _CACHE[key]


def _make_in_maps(inputs, nb=NB, n_cores=N_CORES):
    import ml_dtypes

    BF16 = ml_dtypes.bfloat16
    F8 = ml_dtypes.float8_e4m3

    def q8(a):
        return np.clip(a, -240.0, 240.0).astype(F8)

    x = np.ascontiguousarray(np.asarray(inputs["x"], dtype=np.float32))
    h = np.ascontiguousarray(np.asarray(inputs["h"], dtype=np.float32))
    shared = {}
    for g in ("r", "z", "n"):
        for m in ("q", "k"):
            w = np.asarray(inputs[f"W{m}_{g}"], dtype=np.float32)
            shared[f"W{m}_{g}"] = np.ascontiguousarray(w[D_IN:].astype(BF16))
            shared[f"W{m}8_{g}"] = np.ascontiguousarray(q8(w[:D_IN]))
        shared[f"Wv8_{g}"] = np.ascontiguousarray(
            q8(np.asarray(inputs[f"Wv_{g}"], dtype=np.float32))
        )
        for nm in ("bq", "bk", "bv"):
            shared[f"{nm}_{g}"] = np.ascontiguousarray(
                np.asarray(inputs[f"{nm}_{g}"], dtype=np.float32)
            )
    shared["gamma"] = np.ascontiguousarray(np.asarray(inputs["gamma"], np.float32))
    shared["beta"] = np.ascontiguousarray(np.asarray(inputs["beta"], np.float32))

    in_maps = []
    for ci in range(n_cores):
        sl = slice(ci * nb, (ci + 1) * nb)
        xs = x[sl]
        hs = h[sl]
        xT = xs.transpose(0, 2, 1)
        hT = hs.transpose(0, 2, 1)
        m = dict(shared)
        m["hTb"] = np.ascontiguousarray(hT.astype(BF16))
        m["x8"] = np.ascontiguousarray(q8(xT))
        m["h8"] = np.ascontiguousarray(q8(hT))
        m["hN"] = hs
        in_maps.append(m)
    return in_maps


def _flags(inputs):
    use_bias = any(
        np.any(np.asarray(inputs[f"{nm}_{g}"]))
        for g in ("r", "z", "n")
        for nm in ("bq", "bk", "bv")
    )
    gamma = np.asarray(inputs["gamma"])
    beta = np.asarray(inputs["beta"])
    use_gb = (not np.allclose(gamma, 1.0)) or bool(np.any(beta))
    return bool(use_bias), bool(use_gb)


def kernel(**inputs):
    from concourse import bass_utils

    use_bias, use_gb = _flags(inputs)
    nc = _get_nc(NB, use_bias, use_gb)
    in_maps = _make_in_maps(inputs)
    res = bass_utils.run_bass_kernel_spmd(nc, in_maps, core_ids=list(range(N_CORES)))
    out = np.concatenate([r["out"] for r in res.results], axis=0)
    return np.ascontiguousarray(out.astype(np.float32))
